# revision 26
# baseline (speedup 1.0000x reference)
"""AWQ W4 grouped-dequant matmul on 8 Trainium2 cores.

y = (x / s) @ (w_q * scales).reshape(OUT, IN).T + bias

Column-parallel sharding: each core owns OUT/8 = 1376 output channels
(padded to 1408 = 11*128), x is replicated. Per core the kernel computes
y_shard^T [1408, 2048] = W'[1408, 4096] @ x[4096, 2048].

Schedule (v6): 8 bf16 o-tiles (cols 0..1023) + 3 fp8 DoubleRow o-tiles
(cols 1024..1407). A DoubleRow matmul takes lhsT [128, 2, 128] fp8 and
rhs [128, 2, 512] fp8 and contracts 256 k-rows into a full [128, 512]
PSUM tile at the SAME per-instruction cost as a bf16 matmul (the cost
is output-free-size cycles), i.e. 2x MAC throughput. 16 pair-matmuls
cover all 32 k-tiles, so a DR o-tile costs half a bf16 o-tile.

  - fp8 numerics: the DR tiles' W8 = e4m3(w_q * scales * 2^9) (range
    [0.512, 179] - entirely e4m3-normal) and x8 = e4m3(x * (16/s));
    the PSUM result is scaled by 2^-13 at eviction (activation
    scale), which also folds the smoothing division into the x8
    conversion. bf16 path folds 1/s into W' as before. Measured
    rel_err 0.0193 (gate 2e-2); fp8 coverage is capped by the error
    budget (e4m3 RTN is ~2.7% per operand), which is why only 3 of
    11 tiles ride the 2x path.
  - Wave-1: per kc one packed [w8 bf-cols | scales bf-cols bf16]
    transfer + the x(0) piece; from kc>=16 the DR columns' packed
    pieces interleave into the stream (and their dequant stts
    interleave into the DVE program order) so the DR pack is fully
    landed ~when the wave ends. DMA ~1.87us/kc worst vs PE 1.73us/kc
    for the first half, converging by the tail.
  - PE order: wave (8 bf16 groups, kc-major, 8 PSUM banks) -> c0 DR
    tiles -> per chunk 1..3: 8 bf16 o-tiles then 3 DR tiles.
  - x(1) streams right after the packs; x(2) after chunk-0's stores
    (xn slot 0 is first written by x(2) since chunk 0's x lives in
    its own region); x(3) after chunk-1's bf16 matmuls retire.
  - PSUM eviction on the Scalar engine as activation(Identity, bias,
    scale) into a 4-slot rotating bf16 buffer; plain DMAs stream out.

The toolchain permits AT MOST ONE semaphore wait per instruction. All
waits are standalone engine instructions; every DMA or compute op
carries only its completion increment. DMA completions may reorder, so
waits only target per-transfer semaphores, terminal values of bulk
chunks, or single-producer engine counters.

Host side does only layout/dtype moves: transpose, pad, shard, byte
packing, bf16/fp8 casts (w_q ints are exact in fp8e4m3).
"""

import os
from contextlib import ExitStack

import numpy as np

# ---- problem constants (hardcoded per contract) ----
OUT, N_GROUPS, GROUP = 11008, 32, 128
IN = N_GROUPS * GROUP  # 4096
TOKENS = 2048
N_CORES = 8
P = 128
O_SHARD = OUT // N_CORES  # 1376
O_PAD = 1408  # 11 * 128
OT = O_PAD // P  # 11 o-tiles
OBF = 8  # bf16 o-tiles (0..7)
ODR = OT - OBF  # 3 o-tiles in fp8 DoubleRow
O_BF = OBF * P  # 1024 bf16 output columns
O_DR = ODR * P  # 384 DR output columns
KT = IN // P  # 32 k-tiles (== quant groups, GROUP == P)
KTP = KT // 2  # 16 DoubleRow k-pairs
TCH = 512  # tokens per chunk == PSUM bank free size (f32)
NT = TOKENS // TCH  # 4 chunks
NB = 8  # psum banks
WAVE = 8  # wave-1 groups (tt=0, all bf16 o-tiles)
XB = 2  # x chunk buffers
PWB = 6  # packed wave staging slots
PDB = 8  # packed DR staging slots
PD_ILV = 8  # pd piece j rides with pack piece kc = j + PD_ILV
NYS = 4  # y eviction slots

PW_BYTES = O_BF + 2 * O_BF  # 3072: w8 | sc bf16
PD_BYTES = O_DR + 2 * O_DR  # 1152: w8 | sc bf16

# fp8 scaling: W8 = e4m3(w*sc*2^9), x8 = e4m3(x*(1/s)*2^4)
WSH = 512.0
XSH = 16.0
EV_SCALE = 2.0 ** -13

# post-wave schedule: per chunk, bf16 o-tiles then DR tiles.
# (c0's bf16 tiles all ran in wave 1.)
SCHED = []
for _tt in range(NT):
    for _ot in range(OBF):
        if _tt == 0:
            continue
        SCHED.append(("bf", _tt, _ot))
    for _j in range(ODR):
        SCHED.append(("dr", _tt, _j))
NENT = WAVE + len(SCHED)  # 8 + 36 = 44 evict entries

# matmul count after each entry's last matmul (wave entries are
# kc-major interleaved: group b's last mm is 8*(KT-1)+b+1)
MM_END = [WAVE * (KT - 1) + b + 1 for b in range(WAVE)]
_cum = WAVE * KT  # 256
MM_WAVE_END = _cum
MM_C0_END = MM_C1BF_END = MM_C1_END = None
for _kind, _tt, _i in SCHED:
    _cum += KT if _kind == "bf" else KTP
    MM_END.append(_cum)
    if _tt == 0:
        MM_C0_END = _cum
    if _tt == 1 and _kind == "bf":
        MM_C1BF_END = _cum
    if _tt == 1:
        MM_C1_END = _cum

# ---- DVE (vector engine) program order and s_dq values ----
# recip, inv16, then per kc: bf-stt(kc) (+ dr-stt(kc-PD_ILV) once the
# interleaved DR pack pieces land), then the dr-stt tail, then
# conv-c1/2/3 blocks. The c0 x8 conversions run on the Scalar engine
# (s_cv0) - with them on the DVE the wave becomes DVE-paced (bf-stt
# [128,1024] is ~1.29us; no DVE 2x mode applies because the fp8
# operand is 1-byte).
DQ_BF = {}
DQ_DR = {}
_ctr = 2  # 1 recip, 2 inv16
for _kc in range(KT):
    _ctr += 1
    DQ_BF[_kc] = _ctr
    if _kc >= PD_ILV:
        _ctr += 1
        DQ_DR[_kc - PD_ILV] = _ctr
for _j in range(KT - PD_ILV, KT):
    _ctr += 1
    DQ_DR[_j] = _ctr
DQ_ALL_DR = _ctr  # 66: w8d2 fully dequanted


def dq_conv(tt):
    return DQ_ALL_DR + KT * tt  # chunk-tt x8 conversions done


LAST = {}  # exec_time_ns etc. for the local test harness

_NC_CACHE = {}


def _build_nc():
    import concourse.bass as bass
    from concourse import mybir

    f32 = mybir.dt.float32
    bf16 = mybir.dt.bfloat16
    fp8 = mybir.dt.float8e4
    u8 = mybir.dt.uint8

    nc = bass.Bass()
    pack_w = nc.declare_dram_parameter("pack_w", [IN, PW_BYTES], u8, isOutput=False)
    pack_d = nc.declare_dram_parameter("pack_d", [IN, PD_BYTES], u8, isOutput=False)
    xT = nc.declare_dram_parameter("xT", [IN, TOKENS], bf16, isOutput=False)
    s_cols = nc.declare_dram_parameter("s_cols", [P, KT], f32, isOutput=False)
    bias_cols = nc.declare_dram_parameter("bias_cols", [P, OBF], f32, isOutput=False)
    bias_dr = nc.declare_dram_parameter("bias_dr", [P, ODR], f32, isOutput=False)
    yT = nc.declare_dram_parameter("yT", [O_PAD, TOKENS], bf16, isOutput=True)

    with ExitStack() as ctx:
        w_all = ctx.enter_context(nc.sbuf_tensor("w_all", [P, KT * O_BF], bf16))
        w8d2 = ctx.enter_context(nc.sbuf_tensor("w8d2", [P, KTP, 2, O_DR], fp8))
        xn_all = ctx.enter_context(nc.sbuf_tensor("xn_all", [P, XB * KT * TCH], bf16))
        x8 = ctx.enter_context(nc.sbuf_tensor("x8", [P, XB * KT, TCH], fp8))
        y_sl = ctx.enter_context(nc.sbuf_tensor("y_sl", [P, NYS * TCH], bf16))
        pw = ctx.enter_context(nc.sbuf_tensor("pw", [P, PWB * PW_BYTES], u8))
        pd = ctx.enter_context(nc.sbuf_tensor("pd", [P, PDB * PD_BYTES], u8))
        s_sb = ctx.enter_context(nc.sbuf_tensor("s_sb", [P, KT], f32))
        inv_s = ctx.enter_context(nc.sbuf_tensor("inv_s", [P, KT], f32))
        inv16 = ctx.enter_context(nc.sbuf_tensor("inv16", [P, KT], f32))
        bias_sb = ctx.enter_context(nc.sbuf_tensor("bias_sb", [P, OBF], f32))
        bias_dsb = ctx.enter_context(nc.sbuf_tensor("bias_dsb", [P, ODR], f32))
        ps = [
            ctx.enter_context(nc.psum_tensor(f"ps{i}", [P, TCH], f32))
            for i in range(NB)
        ]
        s_tbl = ctx.enter_context(nc.semaphore("s_tbl"))
        s_wd = [ctx.enter_context(nc.semaphore(f"s_wd{k}")) for k in range(KT)]
        s_x0e = [ctx.enter_context(nc.semaphore(f"s_x0e{q}")) for q in range(KT // 4)]
        s_wdr = [ctx.enter_context(nc.semaphore(f"s_wdr{k}")) for k in range(KT)]
        s_xc = [ctx.enter_context(nc.semaphore(f"s_xc{t}")) for t in range(1, NT)]
        s_dq = ctx.enter_context(nc.semaphore("s_dq"))
        s_cv0 = ctx.enter_context(nc.semaphore("s_cv0"))
        s_pe = ctx.enter_context(nc.semaphore("s_pe"))
        s_act = ctx.enter_context(nc.semaphore("s_act"))
        s_ys = [ctx.enter_context(nc.semaphore(f"s_ys{j}")) for j in range(NYS)]
        block = ctx.enter_context(nc.Block())

        def wd(kc):
            return w_all[:, kc * O_BF : (kc + 1) * O_BF]

        def pw_slot(kc):
            o = (kc % PWB) * PW_BYTES
            return pw[:, o : o + PW_BYTES]

        def pw_w8(kc):
            o = (kc % PWB) * PW_BYTES
            return pw[:, o : o + O_BF].bitcast(mybir.dt.float8e4)

        def pw_sc(kc):
            o = (kc % PWB) * PW_BYTES
            return pw[:, o + O_BF : o + 3 * O_BF].bitcast(bf16)

        def pd_slot(kc):
            o = (kc % PDB) * PD_BYTES
            return pd[:, o : o + PD_BYTES]

        def pd_w8(kc):
            o = (kc % PDB) * PD_BYTES
            return pd[:, o : o + O_DR].bitcast(mybir.dt.float8e4)

        def pd_sc(kc):
            o = (kc % PDB) * PD_BYTES
            return pd[:, o + O_DR : o + PD_BYTES].bitcast(bf16)

        def xreg(tt, kc):
            o = ((tt % XB) * KT + kc) * TCH
            return xn_all[:, o : o + TCH]

        def x8reg(tt, kc):
            return x8[:, (tt % XB) * KT + kc, :]

        def x8pair(tt, kp):
            b = (tt % XB) * KT
            return x8[:, b + 2 * kp : b + 2 * kp + 2, :]

        def yslot(e):
            o = (e % NYS) * TCH
            return y_sl[:, o : o + TCH]

        @block.sync
        def _(sync):
            def xchunk(tt):
                o = (tt % XB) * KT * TCH
                sync.dma_start(
                    out=xn_all[:, o : o + KT * TCH],
                    in_=xT[:, tt * TCH : (tt + 1) * TCH].rearrange(
                        "(kc p) t -> p kc t", kc=KT
                    ),
                ).then_inc(s_xc[tt - 1], 16)

            sync.dma_start(out=s_sb[:, :], in_=s_cols[:, :]).then_inc(s_tbl, 16)
            # wave stream: per kc a packed [w8-bf | sc-bf] transfer plus
            # the x(0) piece (both count s_wd[kc], terminal 32); DR pack
            # pieces interleave from kc >= PD_ILV. Slot WAR gates target
            # the DVE counter (bf-stt / dr-stt are the only readers).
            for kc in range(KT):
                if kc >= PWB:
                    sync.wait_ge(s_dq, DQ_BF[kc - PWB])
                sync.dma_start(
                    out=pw_slot(kc), in_=pack_w[kc * P : (kc + 1) * P, :]
                ).then_inc(s_wd[kc], 16)
                sync.dma_start(
                    out=xreg(0, kc), in_=xT[kc * P : (kc + 1) * P, 0:TCH]
                ).then_inc(s_wd[kc], 16)
                if kc == 26:
                    sync.dma_start(out=bias_sb[:, :], in_=bias_cols[:, :]).then_inc(
                        s_tbl, 16
                    )
                    sync.dma_start(out=bias_dsb[:, :], in_=bias_dr[:, :]).then_inc(
                        s_tbl, 16
                    )
                if kc >= PD_ILV:
                    j = kc - PD_ILV
                    if j >= PDB:
                        sync.wait_ge(s_dq, DQ_DR[j - PDB])
                    sync.dma_start(
                        out=pd_slot(j), in_=pack_d[j * P : (j + 1) * P, :]
                    ).then_inc(s_wdr[j], 16)
            for j in range(KT - PD_ILV, KT):
                if j >= PDB:
                    sync.wait_ge(s_dq, DQ_DR[j - PDB])
                sync.dma_start(
                    out=pd_slot(j), in_=pack_d[j * P : (j + 1) * P, :]
                ).then_inc(s_wdr[j], 16)
            # x chunks 1..3 are terminal-gated, so each ships as ONE
            # batched DMA ([(kc p) t] -> [p, kc, t]) - 1 SP issue instead
            # of 32 (the SP sequencer spends 565ns per dma_start).
            # x(1): needed when chunk-1 bf16 starts (after c0 DR tiles)
            xchunk(1)

            def ystore(e):
                sync.wait_ge(s_act, e + 1)
                if e < WAVE:
                    tt, i = 0, e
                else:
                    kind, tt, i = SCHED[e - WAVE]
                    if kind == "dr":
                        i += OBF
                sync.dma_start(
                    out=yT[i * P : (i + 1) * P, tt * TCH : (tt + 1) * TCH],
                    in_=yslot(e),
                ).then_inc(s_ys[e % NYS], 16)

            NE = WAVE + ODR  # 11 entries for chunk 0, 11 per later chunk
            for e in range(NE):  # c0 stores
                ystore(e)
            # x(2) reuses xn slot 0: wave matmuls and c0 conversions must
            # have retired (both single-producer counters).
            sync.wait_ge(s_pe, MM_WAVE_END)
            sync.wait_ge(s_cv0, KT)
            xchunk(2)
            for e in range(NE, 2 * NE):  # c1 stores
                ystore(e)
            # x(3) reuses xn slot 1: chunk-1 bf16 matmuls and the c1
            # conversions must have retired.
            sync.wait_ge(s_pe, MM_C1BF_END)
            sync.wait_ge(s_dq, dq_conv(1))
            xchunk(3)
            for e in range(2 * NE, NENT):
                ystore(e)
            for j in range(NYS):
                sync.wait_ge(s_ys[j], 16 * (NENT // NYS))

        @block.vector
        def _(vector):
            vector.wait_ge(s_tbl, 16)  # s_cols landed
            nc.vector.reciprocal(out=inv_s[:, :], in_=s_sb[:, :]).then_inc(s_dq, 1)
            vector.wait_ge(s_dq, 1)  # recip retired before reads of inv_s
            nc.vector.tensor_scalar_mul(inv16[:, :], inv_s[:, :], XSH).then_inc(
                s_dq, 1
            )
            vector.wait_ge(s_dq, 2)

            def dr_stt(j):
                vector.wait_ge(s_wdr[j], 16)
                nc.vector.scalar_tensor_tensor(
                    w8d2[:, j // 2, j % 2, :],
                    pd_w8(j),
                    WSH,
                    pd_sc(j),
                    mybir.AluOpType.mult,
                    mybir.AluOpType.mult,
                ).then_inc(s_dq, 1)

            # per kc: dequant bf16 W' and (once the interleaved DR pack
            # pieces land) dequant DR columns
            for kc in range(KT):
                vector.wait_ge(s_wd[kc], 32)
                nc.vector.scalar_tensor_tensor(
                    wd(kc),
                    pw_w8(kc),
                    inv_s[:, kc : kc + 1],
                    pw_sc(kc),
                    mybir.AluOpType.mult,
                    mybir.AluOpType.mult,
                ).then_inc(s_dq, 1)
                if kc >= PD_ILV:
                    dr_stt(kc - PD_ILV)
            for j in range(KT - PD_ILV, KT):
                dr_stt(j)
            # x -> fp8 conversions for chunks 1..3. s_xc is a bulk-chunk
            # counter whose DMA completions may reorder, so only its
            # terminal value is meaningful.
            for tt in range(1, NT):
                if tt == 2:
                    vector.wait_ge(s_pe, MM_C0_END)  # x8 slot 0 free
                if tt == 3:
                    vector.wait_ge(s_pe, MM_C1_END)  # x8 slot 1 free
                vector.wait_ge(s_xc[tt - 1], 16)
                for kc in range(KT):
                    nc.vector.tensor_scalar_mul(
                        x8reg(tt, kc), xreg(tt, kc), inv16[:, kc : kc + 1]
                    ).then_inc(s_dq, 1)

        @block.scalar
        def _(scalar):
            # chunk-0 x8 conversions (x8 = e4m3(x * 16/s)) run here so the
            # DVE wave pipeline stays arrival-paced.
            scalar.wait_ge(s_dq, 2)  # inv16 ready
            for kc in range(KT):
                scalar.wait_ge(s_wd[kc], 32)
                nc.scalar.activation(
                    x8reg(0, kc),
                    xreg(0, kc),
                    mybir.ActivationFunctionType.Copy,
                    bias=0.0,
                    scale=inv16[:, kc : kc + 1],
                ).then_inc(s_cv0, 1)
            scalar.wait_ge(s_tbl, 48)
            for e in range(NENT):
                if e >= NYS:
                    scalar.wait_ge(s_ys[e % NYS], 16 * (e // NYS))
                scalar.wait_ge(s_pe, MM_END[e])
                if e < WAVE:
                    kind, i = "bf", e
                else:
                    kind, _tt, i = SCHED[e - WAVE]
                if kind == "bf":
                    nc.scalar.activation(
                        yslot(e),
                        ps[e % NB][:, :],
                        mybir.ActivationFunctionType.Identity,
                        bias=bias_sb[:, i : i + 1],
                        scale=1.0,
                    ).then_inc(s_act, 1)
                else:
                    nc.scalar.activation(
                        yslot(e),
                        ps[e % NB][:, :],
                        mybir.ActivationFunctionType.Identity,
                        bias=bias_dsb[:, i : i + 1],
                        scale=EV_SCALE,
                    ).then_inc(s_act, 1)

        @block.tensor
        def _(tensor):
            DR = mybir.MatmulPerfMode.DoubleRow
            # wave 1: groups (tt=0, ot=0..7) accumulate kc-major.
            for kc in range(KT):
                tensor.wait_ge(s_wd[kc], 32)  # x(0) piece landed
                tensor.wait_ge(s_dq, DQ_BF[kc])  # W'(kc) dequanted
                for b in range(WAVE):
                    nc.tensor.matmul(
                        ps[b][:, :],
                        wd(kc)[:, b * P : (b + 1) * P],
                        xreg(0, kc),
                        start=(kc == 0),
                        stop=(kc == KT - 1),
                    ).then_inc(s_pe, 1)
            # post-wave entries, sequential. DR tiles are full 128-row
            # DoubleRow groups: 16 pair-matmuls cover all 32 k-tiles.
            for ei, (kind, tt, i) in enumerate(SCHED):
                e = WAVE + ei
                if kind == "bf":
                    if i == 0:
                        tensor.wait_ge(s_xc[tt - 1], 16)
                else:
                    if i == 0:
                        if tt == 0:
                            # chunk-0 DR starts right at wave end and
                            # chases the dr-stt tail with per-pair waits
                            tensor.wait_ge(s_cv0, KT)
                        else:
                            # w8d2 complete and chunk-tt x8 ready
                            tensor.wait_ge(s_dq, dq_conv(tt))
                tensor.wait_ge(s_act, e - NB + 1)  # psum bank recycled
                if kind == "bf":
                    for kc in range(KT):
                        nc.tensor.matmul(
                            ps[e % NB][:, :],
                            wd(kc)[:, i * P : (i + 1) * P],
                            xreg(tt, kc),
                            start=(kc == 0),
                            stop=(kc == KT - 1),
                        ).then_inc(s_pe, 1)
                else:
                    for kp in range(KTP):
                        if tt == 0 and i == 0:
                            tensor.wait_ge(s_dq, DQ_DR[2 * kp + 1])
                        nc.tensor.matmul(
                            ps[e % NB][:, :],
                            w8d2[:, kp, :, i * P : (i + 1) * P],
                            x8pair(tt, kp),
                            start=(kp == 0),
                            stop=(kp == KTP - 1),
                            perf_mode=DR,
                        ).then_inc(s_pe, 1)

    return nc


def get_nc():
    if "nc" not in _NC_CACHE:
        _NC_CACHE["nc"] = _build_nc()
    return _NC_CACHE["nc"]


def _prep_inputs(x, w_q, scales, s, bias):
    import ml_dtypes

    bf16 = ml_dtypes.bfloat16
    fp8 = ml_dtypes.float8_e4m3
    x = np.asarray(x, dtype=np.float32)
    w_q = np.asarray(w_q)
    scales = np.asarray(scales, dtype=np.float32)
    s = np.asarray(s, dtype=np.float32)
    bias = np.asarray(bias, dtype=np.float32)

    pad = O_PAD - O_SHARD  # 32 rows of zero-padding per shard
    w = w_q.reshape(OUT, IN).astype(fp8)  # ints in [-7,7] -> exact
    sc = scales.reshape(OUT, N_GROUPS)  # f32

    xT = np.ascontiguousarray(x.T.astype(bf16))  # [IN, TOKENS] bf16
    s_cols = np.ascontiguousarray(s.reshape(KT, P).T)  # [128, 32] f32

    in_maps = []
    for c in range(N_CORES):
        lo, hi = c * O_SHARD, (c + 1) * O_SHARD
        w_c = np.pad(w[lo:hi], ((0, pad), (0, 0)))  # [O_PAD, IN] fp8
        sc_c = np.pad(sc[lo:hi], ((0, pad), (0, 0)))  # [O_PAD, 32]
        b_c = np.pad(bias[lo:hi], (0, pad))  # [O_PAD]
        w8T = np.ascontiguousarray(w_c.T)  # [IN, O_PAD] fp8
        # scales replicated per k-row (layout move): row k holds
        # sc_c[:, k//128] in bf16
        scT = np.ascontiguousarray(
            np.repeat(sc_c.T.astype(bf16), P, axis=0)
        )  # [IN, O_PAD] bf16
        pack_w = np.concatenate(
            [
                w8T[:, :O_BF].view(np.uint8),
                np.ascontiguousarray(scT[:, :O_BF]).view(np.uint8),
            ],
            axis=1,
        )  # [IN, 3072] u8
        pack_d = np.concatenate(
            [
                w8T[:, O_BF:].view(np.uint8),
                np.ascontiguousarray(scT[:, O_BF:]).view(np.uint8),
            ],
            axis=1,
        )  # [IN, 1152] u8
        in_maps.append(
            {
                "pack_w": np.ascontiguousarray(pack_w),
                "pack_d": np.ascontiguousarray(pack_d),
                "xT": xT,
                "s_cols": s_cols,
                "bias_cols": np.ascontiguousarray(
                    b_c[:O_BF].reshape(OBF, P).T
                ),  # [128, 8] f32
                "bias_dr": np.ascontiguousarray(
                    b_c[O_BF:].reshape(ODR, P).T
                ),  # [128, 3] f32
            }
        )
    return in_maps


def _install_profile_shim():
    """Provide antenv.axon_hooks (NTFF profiling via libaxon ctypes) when
    the container image lacks it. Only used for local perf iteration."""
    import contextlib
    import ctypes
    import sys
    import types

    if "antenv.axon_hooks" in sys.modules:
        return
    so_path = "/opt/axon/libaxon_pjrt.so"
    try:
        lib = ctypes.CDLL(so_path)
    except OSError:
        return
    if not hasattr(lib, "axon_start_nrt_profile"):
        return
    lib.axon_start_nrt_profile.argtypes = [
        ctypes.POINTER(ctypes.c_int64),
        ctypes.c_size_t,
    ]
    lib.axon_start_nrt_profile.restype = ctypes.c_int64
    lib.axon_stop_nrt_profile.argtypes = [ctypes.c_char_p]
    lib.axon_stop_nrt_profile.restype = ctypes.c_int64

    @contextlib.contextmanager
    def _hook(output_dir, device_ids):
        import jax

        jax.devices()
        if device_ids:
            ids = (ctypes.c_int64 * len(device_ids))(*device_ids)
            rc = lib.axon_start_nrt_profile(ids, len(device_ids))
        else:
            rc = lib.axon_start_nrt_profile(None, 0)
        if rc != 0:
            raise RuntimeError(f"axon_start_nrt_profile rc={rc}")
        try:
            yield
        finally:
            n = lib.axon_stop_nrt_profile(str(output_dir).encode())
            print(f"profile: {n} file(s) written to {output_dir}", file=sys.stderr)

    mod = types.ModuleType("antenv.axon_hooks")
    mod.get_axon_ntff_profile_hook = lambda: _hook
    mod.set_axon_ntff_profile_hook = lambda h: None
    sys.modules["antenv.axon_hooks"] = mod


def kernel(x, w_q, scales, s, bias):
    import sys

    if "/opt/trn_rl_repo" not in sys.path:
        sys.path.insert(0, "/opt/trn_rl_repo")
    import concourse.bass_utils as bass_utils
    from concourse.bass_utils import run_bass_kernel_spmd

    orig_dtype = np.asarray(x).dtype
    in_maps = _prep_inputs(x, w_q, scales, s, bias)
    nc = get_nc()

    trace = bool(os.environ.get("AWQ_TRACE"))
    kwargs = {}
    if trace:
        _install_profile_shim()
        bass_utils.upload_artifacts = lambda d: d  # zero-egress container
        tmpdir = os.environ.get("AWQ_TRACE_DIR")
        if tmpdir:
            os.makedirs(tmpdir, exist_ok=True)
            kwargs["tmpdir"] = tmpdir
    res = run_bass_kernel_spmd(
        nc,
        in_maps,
        core_ids=list(range(N_CORES)),
        trace=trace,
        **kwargs,
    )
    LAST["exec_time_ns"] = res.exec_time_ns
    LAST["results"] = res

    yT_full = np.concatenate(
        [np.asarray(res.results[c]["yT"], dtype=np.float32) for c in range(N_CORES)],
        axis=0,
    )  # [8*1408, 2048] f32
    y = np.ascontiguousarray(
        yT_full.reshape(N_CORES, O_PAD, TOKENS)[:, :O_SHARD, :]
        .reshape(OUT, TOKENS)
        .T
    )
    return y.astype(orig_dtype)


# revision 27
# speedup vs baseline: 1.0467x; 1.0467x over previous
"""AWQ W4 grouped-dequant matmul on 8 Trainium2 cores.

y = (x / s) @ (w_q * scales).reshape(OUT, IN).T + bias

Column-parallel sharding: each core owns OUT/8 = 1376 output channels
(padded to 1408 = 11*128), x is replicated. Per core the kernel computes
y_shard^T [1408, 2048] = W'[1408, 4096] @ x[4096, 2048].

Schedule (v11): 8 bf16 o-tiles (cols 0..1023) + 3 fp8 DoubleRow o-tiles
(cols 1024..1407). A DoubleRow matmul takes lhsT [128, 2, 128] fp8 and
rhs [128, 2, 512] fp8 and contracts 256 k-rows into a full [128, 512]
PSUM tile at the SAME per-instruction cost as a bf16 matmul (the cost
is output-free-size cycles), i.e. 2x MAC throughput. 16 pair-matmuls
cover all 32 k-tiles, so a DR o-tile costs half a bf16 o-tile.

  - fp8 numerics: the DR tiles' W8 = e4m3(w_q * scales * 2^9) (range
    [0.512, 179] - entirely e4m3-normal) and x8 = e4m3(x * (16/s));
    the PSUM result is scaled by 2^-13 at eviction (activation
    scale), which also folds the smoothing division into the x8
    conversion. bf16 path folds 1/s into W' as before. Measured
    rel_err 0.0193 (gate 2e-2); fp8 coverage is capped by the error
    budget (e4m3 RTN is ~2.7% per operand), which is why only 3 of
    11 tiles ride the 2x path.
  - Wave-1: per kc one packed [w8 bf-cols | scales bf-cols bf16]
    transfer + the x(0) piece; from kc>=PD_ILV the DR columns' packed
    pieces interleave into the stream (and their dequant stts into
    the DVE program order) so the DR pack mostly lands by wave end.
  - Engine split: DVE does the W' dequants only (bf-stt [128,1024] is
    ~1.29us - with the x8 conversions it would pace the wave); the
    Scalar engine does chunk-0's x8 conversions (activation Copy with
    per-partition scale 16/s) and all PSUM evictions; chunk 1..3
    conversions ride the post-wave DVE slack.
  - PE order: wave (8 bf16 groups, kc-major, 8 PSUM banks) -> c0 DR
    tiles (tile 0 chases the dr-stt tail with per-pair s_dq waits) ->
    per chunk 1..3: 8 bf16 o-tiles then 3 DR tiles.
  - x(1)/x(2)/x(3) each ship as ONE batched DMA ((kc p) t -> p kc t
    rearrange) - the SP sequencer spends 565ns per dma_start, so the
    per-piece version serializes the front. x(1) follows the packs;
    x(2) goes after chunk-0's stores (xn slot 0 is first written by
    x(2): chunk 0's x has its own region); x(3) after chunk-1's bf16
    matmuls and conversions retire.
  - PSUM eviction on the Scalar engine as activation(Identity, bias,
    scale) into a 4-slot rotating bf16 buffer; plain DMAs stream out.
  - The deployed cost model bills every matmul at output-free-size
    cycles regardless of dtype/perf-mode (DoubleRow gets NO cycle
    discount), so fp8's 2x comes ONLY from the 256-deep contraction
    of 128-row DoubleRow; the front (pack + x0 + DR pack + x1,
    ~25.7MB at ~360GB/s) and the PE (1216 matmuls x 213.3ns) are the
    two binding resources.

The toolchain permits AT MOST ONE semaphore wait per instruction. All
waits are standalone engine instructions; every DMA or compute op
carries only its completion increment. DMA completions may reorder, so
waits only target per-transfer semaphores, terminal values of bulk
chunks, or single-producer engine counters.

Host side does only layout/dtype moves: transpose, pad, shard, byte
packing, bf16/fp8 casts (w_q ints are exact in fp8e4m3).
"""

import os
from contextlib import ExitStack

import numpy as np

# ---- problem constants (hardcoded per contract) ----
OUT, N_GROUPS, GROUP = 11008, 32, 128
IN = N_GROUPS * GROUP  # 4096
TOKENS = 2048
N_CORES = 8
P = 128
O_SHARD = OUT // N_CORES  # 1376
O_PAD = 1408  # 11 * 128
OT = O_PAD // P  # 11 o-tiles
OBF = 8  # bf16 o-tiles (0..7)
ODR = OT - OBF  # 3 o-tiles in fp8 DoubleRow
O_BF = OBF * P  # 1024 bf16 output columns
O_DR = ODR * P  # 384 DR output columns
KT = IN // P  # 32 k-tiles (== quant groups, GROUP == P)
KTP = KT // 2  # 16 DoubleRow k-pairs
TCH = 512  # tokens per chunk == PSUM bank free size (f32)
NT = TOKENS // TCH  # 4 chunks
NB = 8  # psum banks
WAVE = 8  # wave-1 groups (tt=0, all bf16 o-tiles)
XB = 2  # x chunk buffers
PWB = 6  # packed wave staging slots
PDB = 8  # packed DR staging slots
PD_ILV = 8  # pd piece j rides with pack piece kc = j + PD_ILV
NYS = 4  # y eviction slots

PW_BYTES = O_BF + 2 * O_BF  # 3072: w8 | sc bf16
PD_BYTES = O_DR + 2 * O_DR  # 1152: w8 | sc bf16

# fp8 scaling: W8 = e4m3(w*sc*2^9), x8 = e4m3(x*(1/s)*2^4)
WSH = 512.0
XSH = 16.0
EV_SCALE = 2.0 ** -13

# post-wave schedule: per chunk, bf16 o-tiles then DR tiles.
# (c0's bf16 tiles all ran in wave 1.)
SCHED = []
for _tt in range(NT):
    for _ot in range(OBF):
        if _tt == 0:
            continue
        SCHED.append(("bf", _tt, _ot))
    for _j in range(ODR):
        SCHED.append(("dr", _tt, _j))
NENT = WAVE + len(SCHED)  # 8 + 36 = 44 evict entries

# matmul count after each entry's last matmul (wave entries are
# kc-major interleaved: group b's last mm is 8*(KT-1)+b+1)
MM_END = [WAVE * (KT - 1) + b + 1 for b in range(WAVE)]
_cum = WAVE * KT  # 256
MM_WAVE_END = _cum
MM_C0_END = MM_C1BF_END = MM_C1_END = None
for _kind, _tt, _i in SCHED:
    _cum += KT if _kind == "bf" else KTP
    MM_END.append(_cum)
    if _tt == 0:
        MM_C0_END = _cum
    if _tt == 1 and _kind == "bf":
        MM_C1BF_END = _cum
    if _tt == 1:
        MM_C1_END = _cum

# ---- DVE (vector engine) program order and s_dq values ----
# recip, inv16, then per kc: bf-stt(kc) (+ dr-stt(kc-PD_ILV) once the
# interleaved DR pack pieces land), then the dr-stt tail, then
# conv-c1/2/3 blocks. The c0 x8 conversions run on the Scalar engine
# (s_cv0) - with them on the DVE the wave becomes DVE-paced (bf-stt
# [128,1024] is ~1.29us; no DVE 2x mode applies because the fp8
# operand is 1-byte).
DQ_BF = {}
DQ_DR = {}
_ctr = 2  # 1 recip, 2 inv16
for _kc in range(KT):
    _ctr += 1
    DQ_BF[_kc] = _ctr
    if _kc >= PD_ILV:
        _ctr += 1
        DQ_DR[_kc - PD_ILV] = _ctr
for _j in range(KT - PD_ILV, KT):
    _ctr += 1
    DQ_DR[_j] = _ctr
DQ_ALL_DR = _ctr  # 66: w8d2 fully dequanted


def dq_conv(tt):
    return DQ_ALL_DR + KT * tt  # chunk-tt x8 conversions done


LAST = {}  # exec_time_ns etc. for the local test harness

_NC_CACHE = {}


def _build_nc():
    import concourse.bass as bass
    from concourse import mybir

    f32 = mybir.dt.float32
    bf16 = mybir.dt.bfloat16
    fp8 = mybir.dt.float8e4
    u8 = mybir.dt.uint8

    nc = bass.Bass()
    pack_w = nc.declare_dram_parameter("pack_w", [IN, PW_BYTES], u8, isOutput=False)
    pack_d = nc.declare_dram_parameter("pack_d", [IN, PD_BYTES], u8, isOutput=False)
    xT = nc.declare_dram_parameter("xT", [IN, TOKENS], bf16, isOutput=False)
    s_cols = nc.declare_dram_parameter("s_cols", [P, KT], f32, isOutput=False)
    bias_cols = nc.declare_dram_parameter("bias_cols", [P, OBF], f32, isOutput=False)
    bias_dr = nc.declare_dram_parameter("bias_dr", [P, ODR], f32, isOutput=False)
    yT = nc.declare_dram_parameter("yT", [O_PAD, TOKENS], bf16, isOutput=True)

    with ExitStack() as ctx:
        w_all = ctx.enter_context(nc.sbuf_tensor("w_all", [P, KT * O_BF], bf16))
        w8d2 = ctx.enter_context(nc.sbuf_tensor("w8d2", [P, KTP, 2, O_DR], fp8))
        xn_all = ctx.enter_context(nc.sbuf_tensor("xn_all", [P, XB * KT * TCH], bf16))
        x8 = ctx.enter_context(nc.sbuf_tensor("x8", [P, XB * KT, TCH], fp8))
        y_sl = ctx.enter_context(nc.sbuf_tensor("y_sl", [P, NYS * TCH], bf16))
        pw = ctx.enter_context(nc.sbuf_tensor("pw", [P, PWB * PW_BYTES], u8))
        pd = ctx.enter_context(nc.sbuf_tensor("pd", [P, PDB * PD_BYTES], u8))
        s_sb = ctx.enter_context(nc.sbuf_tensor("s_sb", [P, KT], f32))
        inv_s = ctx.enter_context(nc.sbuf_tensor("inv_s", [P, KT], f32))
        inv16 = ctx.enter_context(nc.sbuf_tensor("inv16", [P, KT], f32))
        bias_sb = ctx.enter_context(nc.sbuf_tensor("bias_sb", [P, OBF], f32))
        bias_dsb = ctx.enter_context(nc.sbuf_tensor("bias_dsb", [P, ODR], f32))
        ps = [
            ctx.enter_context(nc.psum_tensor(f"ps{i}", [P, TCH], f32))
            for i in range(NB)
        ]
        s_tbl = ctx.enter_context(nc.semaphore("s_tbl"))
        s_wd = [ctx.enter_context(nc.semaphore(f"s_wd{k}")) for k in range(KT)]
        s_wdr = [ctx.enter_context(nc.semaphore(f"s_wdr{k}")) for k in range(KT)]
        s_xc = [ctx.enter_context(nc.semaphore(f"s_xc{t}")) for t in range(1, NT)]
        s_dq = ctx.enter_context(nc.semaphore("s_dq"))
        s_cv0 = ctx.enter_context(nc.semaphore("s_cv0"))
        s_pe = ctx.enter_context(nc.semaphore("s_pe"))
        s_act = ctx.enter_context(nc.semaphore("s_act"))
        s_ys = [ctx.enter_context(nc.semaphore(f"s_ys{j}")) for j in range(NYS)]
        block = ctx.enter_context(nc.Block())

        def wd(kc):
            return w_all[:, kc * O_BF : (kc + 1) * O_BF]

        def pw_slot(kc):
            o = (kc % PWB) * PW_BYTES
            return pw[:, o : o + PW_BYTES]

        def pw_w8(kc):
            o = (kc % PWB) * PW_BYTES
            return pw[:, o : o + O_BF].bitcast(mybir.dt.float8e4)

        def pw_sc(kc):
            o = (kc % PWB) * PW_BYTES
            return pw[:, o + O_BF : o + 3 * O_BF].bitcast(bf16)

        def pd_slot(kc):
            o = (kc % PDB) * PD_BYTES
            return pd[:, o : o + PD_BYTES]

        def pd_w8(kc):
            o = (kc % PDB) * PD_BYTES
            return pd[:, o : o + O_DR].bitcast(mybir.dt.float8e4)

        def pd_sc(kc):
            o = (kc % PDB) * PD_BYTES
            return pd[:, o + O_DR : o + PD_BYTES].bitcast(bf16)

        def xreg(tt, kc):
            o = ((tt % XB) * KT + kc) * TCH
            return xn_all[:, o : o + TCH]

        def x8reg(tt, kc):
            return x8[:, (tt % XB) * KT + kc, :]

        def x8pair(tt, kp):
            b = (tt % XB) * KT
            return x8[:, b + 2 * kp : b + 2 * kp + 2, :]

        def yslot(e):
            o = (e % NYS) * TCH
            return y_sl[:, o : o + TCH]

        @block.sync
        def _(sync):
            def xchunk(tt):
                o = (tt % XB) * KT * TCH
                sync.dma_start(
                    out=xn_all[:, o : o + KT * TCH],
                    in_=xT[:, tt * TCH : (tt + 1) * TCH].rearrange(
                        "(kc p) t -> p kc t", kc=KT
                    ),
                ).then_inc(s_xc[tt - 1], 16)

            sync.dma_start(out=s_sb[:, :], in_=s_cols[:, :]).then_inc(s_tbl, 16)
            # wave stream: per kc a packed [w8-bf | sc-bf] transfer plus
            # the x(0) piece (both count s_wd[kc], terminal 32); DR pack
            # pieces interleave from kc >= PD_ILV. Slot WAR gates target
            # the DVE counter (bf-stt / dr-stt are the only readers).
            for kc in range(KT):
                if kc >= PWB:
                    sync.wait_ge(s_dq, DQ_BF[kc - PWB])
                sync.dma_start(
                    out=pw_slot(kc), in_=pack_w[kc * P : (kc + 1) * P, :]
                ).then_inc(s_wd[kc], 16)
                sync.dma_start(
                    out=xreg(0, kc), in_=xT[kc * P : (kc + 1) * P, 0:TCH]
                ).then_inc(s_wd[kc], 16)
                if kc == 26:
                    sync.dma_start(out=bias_sb[:, :], in_=bias_cols[:, :]).then_inc(
                        s_tbl, 16
                    )
                    sync.dma_start(out=bias_dsb[:, :], in_=bias_dr[:, :]).then_inc(
                        s_tbl, 16
                    )
                if kc >= PD_ILV:
                    j = kc - PD_ILV
                    if j >= PDB:
                        sync.wait_ge(s_dq, DQ_DR[j - PDB])
                    sync.dma_start(
                        out=pd_slot(j), in_=pack_d[j * P : (j + 1) * P, :]
                    ).then_inc(s_wdr[j], 16)
            for j in range(KT - PD_ILV, KT):
                if j >= PDB:
                    sync.wait_ge(s_dq, DQ_DR[j - PDB])
                sync.dma_start(
                    out=pd_slot(j), in_=pack_d[j * P : (j + 1) * P, :]
                ).then_inc(s_wdr[j], 16)
            # x chunks 1..3 are terminal-gated, so each ships as ONE
            # batched DMA ([(kc p) t] -> [p, kc, t]) - 1 SP issue instead
            # of 32 (the SP sequencer spends 565ns per dma_start).
            # x(1): needed when chunk-1 bf16 starts (after c0 DR tiles)
            xchunk(1)

            def ystore(e):
                sync.wait_ge(s_act, e + 1)
                if e < WAVE:
                    tt, i = 0, e
                else:
                    kind, tt, i = SCHED[e - WAVE]
                    if kind == "dr":
                        i += OBF
                sync.dma_start(
                    out=yT[i * P : (i + 1) * P, tt * TCH : (tt + 1) * TCH],
                    in_=yslot(e),
                ).then_inc(s_ys[e % NYS], 16)

            NE = WAVE + ODR  # 11 entries for chunk 0, 11 per later chunk
            for e in range(NE):  # c0 stores
                ystore(e)
            # x(2) reuses xn slot 0: wave matmuls and c0 conversions must
            # have retired (both single-producer counters).
            sync.wait_ge(s_pe, MM_WAVE_END)
            sync.wait_ge(s_cv0, KT)
            xchunk(2)
            for e in range(NE, 2 * NE):  # c1 stores
                ystore(e)
            # x(3) reuses xn slot 1: chunk-1 bf16 matmuls and the c1
            # conversions must have retired.
            sync.wait_ge(s_pe, MM_C1BF_END)
            sync.wait_ge(s_dq, dq_conv(1))
            xchunk(3)
            for e in range(2 * NE, NENT):
                ystore(e)
            for j in range(NYS):
                sync.wait_ge(s_ys[j], 16 * (NENT // NYS))

        @block.vector
        def _(vector):
            vector.wait_ge(s_tbl, 16)  # s_cols landed
            nc.vector.reciprocal(out=inv_s[:, :], in_=s_sb[:, :]).then_inc(s_dq, 1)
            vector.wait_ge(s_dq, 1)  # recip retired before reads of inv_s
            nc.vector.tensor_scalar_mul(inv16[:, :], inv_s[:, :], XSH).then_inc(
                s_dq, 1
            )
            vector.wait_ge(s_dq, 2)

            def dr_stt(j):
                vector.wait_ge(s_wdr[j], 16)
                nc.vector.scalar_tensor_tensor(
                    w8d2[:, j // 2, j % 2, :],
                    pd_w8(j),
                    WSH,
                    pd_sc(j),
                    mybir.AluOpType.mult,
                    mybir.AluOpType.mult,
                ).then_inc(s_dq, 1)

            # per kc: dequant bf16 W' and (once the interleaved DR pack
            # pieces land) dequant DR columns
            for kc in range(KT):
                vector.wait_ge(s_wd[kc], 32)
                nc.vector.scalar_tensor_tensor(
                    wd(kc),
                    pw_w8(kc),
                    inv_s[:, kc : kc + 1],
                    pw_sc(kc),
                    mybir.AluOpType.mult,
                    mybir.AluOpType.mult,
                ).then_inc(s_dq, 1)
                if kc >= PD_ILV:
                    dr_stt(kc - PD_ILV)
            for j in range(KT - PD_ILV, KT):
                dr_stt(j)
            # x -> fp8 conversions for chunks 1..3. s_xc is a bulk-chunk
            # counter whose DMA completions may reorder, so only its
            # terminal value is meaningful.
            for tt in range(1, NT):
                if tt == 2:
                    vector.wait_ge(s_pe, MM_C0_END)  # x8 slot 0 free
                if tt == 3:
                    vector.wait_ge(s_pe, MM_C1_END)  # x8 slot 1 free
                vector.wait_ge(s_xc[tt - 1], 16)
                for kc in range(KT):
                    nc.vector.tensor_scalar_mul(
                        x8reg(tt, kc), xreg(tt, kc), inv16[:, kc : kc + 1]
                    ).then_inc(s_dq, 1)

        @block.scalar
        def _(scalar):
            # chunk-0 x8 conversions (x8 = e4m3(x * 16/s)) run here so the
            # DVE wave pipeline stays arrival-paced.
            scalar.wait_ge(s_dq, 2)  # inv16 ready
            for kc in range(KT):
                scalar.wait_ge(s_wd[kc], 32)
                nc.scalar.activation(
                    x8reg(0, kc),
                    xreg(0, kc),
                    mybir.ActivationFunctionType.Copy,
                    bias=0.0,
                    scale=inv16[:, kc : kc + 1],
                ).then_inc(s_cv0, 1)
            scalar.wait_ge(s_tbl, 48)
            for e in range(NENT):
                if e >= NYS:
                    scalar.wait_ge(s_ys[e % NYS], 16 * (e // NYS))
                scalar.wait_ge(s_pe, MM_END[e])
                if e < WAVE:
                    kind, i = "bf", e
                else:
                    kind, _tt, i = SCHED[e - WAVE]
                if kind == "bf":
                    nc.scalar.activation(
                        yslot(e),
                        ps[e % NB][:, :],
                        mybir.ActivationFunctionType.Identity,
                        bias=bias_sb[:, i : i + 1],
                        scale=1.0,
                    ).then_inc(s_act, 1)
                else:
                    nc.scalar.activation(
                        yslot(e),
                        ps[e % NB][:, :],
                        mybir.ActivationFunctionType.Identity,
                        bias=bias_dsb[:, i : i + 1],
                        scale=EV_SCALE,
                    ).then_inc(s_act, 1)

        @block.tensor
        def _(tensor):
            DR = mybir.MatmulPerfMode.DoubleRow
            # wave 1: groups (tt=0, ot=0..7) accumulate kc-major.
            for kc in range(KT):
                tensor.wait_ge(s_wd[kc], 32)  # x(0) piece landed
                tensor.wait_ge(s_dq, DQ_BF[kc])  # W'(kc) dequanted
                for b in range(WAVE):
                    nc.tensor.matmul(
                        ps[b][:, :],
                        wd(kc)[:, b * P : (b + 1) * P],
                        xreg(0, kc),
                        start=(kc == 0),
                        stop=(kc == KT - 1),
                    ).then_inc(s_pe, 1)
            # post-wave entries, sequential. DR tiles are full 128-row
            # DoubleRow groups: 16 pair-matmuls cover all 32 k-tiles.
            for ei, (kind, tt, i) in enumerate(SCHED):
                e = WAVE + ei
                if kind == "bf":
                    if i == 0:
                        tensor.wait_ge(s_xc[tt - 1], 16)
                else:
                    if i == 0:
                        if tt == 0:
                            # chunk-0 DR starts right at wave end and
                            # chases the dr-stt tail with per-pair waits
                            tensor.wait_ge(s_cv0, KT)
                        else:
                            # w8d2 complete and chunk-tt x8 ready
                            tensor.wait_ge(s_dq, dq_conv(tt))
                tensor.wait_ge(s_act, e - NB + 1)  # psum bank recycled
                if kind == "bf":
                    for kc in range(KT):
                        nc.tensor.matmul(
                            ps[e % NB][:, :],
                            wd(kc)[:, i * P : (i + 1) * P],
                            xreg(tt, kc),
                            start=(kc == 0),
                            stop=(kc == KT - 1),
                        ).then_inc(s_pe, 1)
                else:
                    for kp in range(KTP):
                        if tt == 0 and i == 0:
                            tensor.wait_ge(s_dq, DQ_DR[2 * kp + 1])
                        nc.tensor.matmul(
                            ps[e % NB][:, :],
                            w8d2[:, kp, :, i * P : (i + 1) * P],
                            x8pair(tt, kp),
                            start=(kp == 0),
                            stop=(kp == KTP - 1),
                            perf_mode=DR,
                        ).then_inc(s_pe, 1)

    return nc


def get_nc():
    if "nc" not in _NC_CACHE:
        _NC_CACHE["nc"] = _build_nc()
    return _NC_CACHE["nc"]


def _prep_inputs(x, w_q, scales, s, bias):
    import ml_dtypes

    bf16 = ml_dtypes.bfloat16
    fp8 = ml_dtypes.float8_e4m3
    x = np.asarray(x, dtype=np.float32)
    w_q = np.asarray(w_q)
    scales = np.asarray(scales, dtype=np.float32)
    s = np.asarray(s, dtype=np.float32)
    bias = np.asarray(bias, dtype=np.float32)

    pad = O_PAD - O_SHARD  # 32 rows of zero-padding per shard
    w = w_q.reshape(OUT, IN).astype(fp8)  # ints in [-7,7] -> exact
    sc = scales.reshape(OUT, N_GROUPS)  # f32

    xT = np.ascontiguousarray(x.T.astype(bf16))  # [IN, TOKENS] bf16
    s_cols = np.ascontiguousarray(s.reshape(KT, P).T)  # [128, 32] f32

    in_maps = []
    for c in range(N_CORES):
        lo, hi = c * O_SHARD, (c + 1) * O_SHARD
        w_c = np.pad(w[lo:hi], ((0, pad), (0, 0)))  # [O_PAD, IN] fp8
        sc_c = np.pad(sc[lo:hi], ((0, pad), (0, 0)))  # [O_PAD, 32]
        b_c = np.pad(bias[lo:hi], (0, pad))  # [O_PAD]
        w8T = np.ascontiguousarray(w_c.T)  # [IN, O_PAD] fp8
        # scales replicated per k-row (layout move): row k holds
        # sc_c[:, k//128] in bf16
        scT = np.ascontiguousarray(
            np.repeat(sc_c.T.astype(bf16), P, axis=0)
        )  # [IN, O_PAD] bf16
        pack_w = np.concatenate(
            [
                w8T[:, :O_BF].view(np.uint8),
                np.ascontiguousarray(scT[:, :O_BF]).view(np.uint8),
            ],
            axis=1,
        )  # [IN, 3072] u8
        pack_d = np.concatenate(
            [
                w8T[:, O_BF:].view(np.uint8),
                np.ascontiguousarray(scT[:, O_BF:]).view(np.uint8),
            ],
            axis=1,
        )  # [IN, 1152] u8
        in_maps.append(
            {
                "pack_w": np.ascontiguousarray(pack_w),
                "pack_d": np.ascontiguousarray(pack_d),
                "xT": xT,
                "s_cols": s_cols,
                "bias_cols": np.ascontiguousarray(
                    b_c[:O_BF].reshape(OBF, P).T
                ),  # [128, 8] f32
                "bias_dr": np.ascontiguousarray(
                    b_c[O_BF:].reshape(ODR, P).T
                ),  # [128, 3] f32
            }
        )
    return in_maps


def _install_profile_shim():
    """Provide antenv.axon_hooks (NTFF profiling via libaxon ctypes) when
    the container image lacks it. Only used for local perf iteration."""
    import contextlib
    import ctypes
    import sys
    import types

    if "antenv.axon_hooks" in sys.modules:
        return
    so_path = "/opt/axon/libaxon_pjrt.so"
    try:
        lib = ctypes.CDLL(so_path)
    except OSError:
        return
    if not hasattr(lib, "axon_start_nrt_profile"):
        return
    lib.axon_start_nrt_profile.argtypes = [
        ctypes.POINTER(ctypes.c_int64),
        ctypes.c_size_t,
    ]
    lib.axon_start_nrt_profile.restype = ctypes.c_int64
    lib.axon_stop_nrt_profile.argtypes = [ctypes.c_char_p]
    lib.axon_stop_nrt_profile.restype = ctypes.c_int64

    @contextlib.contextmanager
    def _hook(output_dir, device_ids):
        import jax

        jax.devices()
        if device_ids:
            ids = (ctypes.c_int64 * len(device_ids))(*device_ids)
            rc = lib.axon_start_nrt_profile(ids, len(device_ids))
        else:
            rc = lib.axon_start_nrt_profile(None, 0)
        if rc != 0:
            raise RuntimeError(f"axon_start_nrt_profile rc={rc}")
        try:
            yield
        finally:
            n = lib.axon_stop_nrt_profile(str(output_dir).encode())
            print(f"profile: {n} file(s) written to {output_dir}", file=sys.stderr)

    mod = types.ModuleType("antenv.axon_hooks")
    mod.get_axon_ntff_profile_hook = lambda: _hook
    mod.set_axon_ntff_profile_hook = lambda h: None
    sys.modules["antenv.axon_hooks"] = mod


def kernel(x, w_q, scales, s, bias):
    import sys

    if "/opt/trn_rl_repo" not in sys.path:
        sys.path.insert(0, "/opt/trn_rl_repo")
    import concourse.bass_utils as bass_utils
    from concourse.bass_utils import run_bass_kernel_spmd

    orig_dtype = np.asarray(x).dtype
    in_maps = _prep_inputs(x, w_q, scales, s, bias)
    nc = get_nc()

    trace = bool(os.environ.get("AWQ_TRACE"))
    kwargs = {}
    if trace:
        _install_profile_shim()
        bass_utils.upload_artifacts = lambda d: d  # zero-egress container
        tmpdir = os.environ.get("AWQ_TRACE_DIR")
        if tmpdir:
            os.makedirs(tmpdir, exist_ok=True)
            kwargs["tmpdir"] = tmpdir
    res = run_bass_kernel_spmd(
        nc,
        in_maps,
        core_ids=list(range(N_CORES)),
        trace=trace,
        **kwargs,
    )
    LAST["exec_time_ns"] = res.exec_time_ns
    LAST["results"] = res

    yT_full = np.concatenate(
        [np.asarray(res.results[c]["yT"], dtype=np.float32) for c in range(N_CORES)],
        axis=0,
    )  # [8*1408, 2048] f32
    y = np.ascontiguousarray(
        yT_full.reshape(N_CORES, O_PAD, TOKENS)[:, :O_SHARD, :]
        .reshape(OUT, TOKENS)
        .T
    )
    return y.astype(orig_dtype)


# revision 30
# speedup vs baseline: 1.1016x; 1.0524x over previous
"""AWQ W4 grouped-dequant matmul on 8 Trainium2 cores.

y = (x / s) @ (w_q * scales).reshape(OUT, IN).T + bias

Column-parallel sharding: each core owns OUT/8 = 1376 output channels
(padded to 1408 = 11*128), x is replicated. Per core the kernel computes
y_shard^T [1408, 2048] = W'[1408, 4096] @ x[4096, 2048].

Schedule (v11): 8 bf16 o-tiles (cols 0..1023) + 3 fp8 DoubleRow o-tiles
(cols 1024..1407). A DoubleRow matmul takes lhsT [128, 2, 128] fp8 and
rhs [128, 2, 512] fp8 and contracts 256 k-rows into a full [128, 512]
PSUM tile at the SAME per-instruction cost as a bf16 matmul (the cost
is output-free-size cycles), i.e. 2x MAC throughput. 16 pair-matmuls
cover all 32 k-tiles, so a DR o-tile costs half a bf16 o-tile.

  - fp8 numerics: the DR tiles' W8 = e4m3(w_q * scales * 2^9) (range
    [0.512, 179] - entirely e4m3-normal) and x8 = e4m3(x * (16/s));
    the PSUM result is scaled by 2^-13 at eviction (activation
    scale), which also folds the smoothing division into the x8
    conversion. bf16 path folds 1/s into W' as before. Measured
    rel_err 0.0193 (gate 2e-2); fp8 coverage is capped by the error
    budget (e4m3 RTN is ~2.7% per operand), which is why only 3 of
    11 tiles ride the 2x path.
  - Wave-1: per kc one packed [w8 bf-cols | scales bf-cols bf16]
    transfer + the x(0) piece; from kc>=PD_ILV the DR columns' packed
    pieces interleave into the stream (and their dequant stts into
    the DVE program order) so the DR pack mostly lands by wave end.
  - Engine split: DVE does the W' dequants only (bf-stt [128,1024] is
    ~1.29us - with the x8 conversions it would pace the wave); the
    Scalar engine does chunk-0's x8 conversions (activation Copy with
    per-partition scale 16/s) and all PSUM evictions; chunk 1..3
    conversions ride the post-wave DVE slack.
  - PE order: wave (8 bf16 groups, kc-major, 8 PSUM banks) -> c0 DR
    tiles (tile 0 chases the dr-stt tail with per-pair s_dq waits) ->
    per chunk 1..3: 8 bf16 o-tiles then 3 DR tiles.
  - x(1)/x(2)/x(3) each ship as ONE batched DMA ((kc p) t -> p kc t
    rearrange) - the SP sequencer spends 565ns per dma_start, so the
    per-piece version serializes the front. x(1) follows the packs;
    x(2) goes after chunk-0's stores (xn slot 0 is first written by
    x(2): chunk 0's x has its own region); x(3) after chunk-1's bf16
    matmuls and conversions retire.
  - PSUM eviction on the Scalar engine as activation(Identity, bias,
    scale) into a 4-slot rotating bf16 buffer; plain DMAs stream out.
  - The deployed cost model bills every matmul at output-free-size
    cycles regardless of dtype/perf-mode (DoubleRow gets NO cycle
    discount), so fp8's 2x comes ONLY from the 256-deep contraction
    of 128-row DoubleRow; the front (pack + x0 + DR pack + x1,
    ~25.7MB at ~360GB/s) and the PE (1216 matmuls x 213.3ns) are the
    two binding resources.

The toolchain permits AT MOST ONE semaphore wait per instruction. All
waits are standalone engine instructions; every DMA or compute op
carries only its completion increment. DMA completions may reorder, so
waits only target per-transfer semaphores, terminal values of bulk
chunks, or single-producer engine counters.

Host side does only layout/dtype moves: transpose, pad, shard, byte
packing, bf16/fp8 casts (w_q ints are exact in fp8e4m3).
"""

import os
from contextlib import ExitStack

import numpy as np

# ---- problem constants (hardcoded per contract) ----
OUT, N_GROUPS, GROUP = 11008, 32, 128
IN = N_GROUPS * GROUP  # 4096
TOKENS = 2048
N_CORES = 8
P = 128
O_SHARD = OUT // N_CORES  # 1376
O_PAD = 1408  # 11 * 128
OT = O_PAD // P  # 11 o-tiles
OBF = 8  # bf16 o-tiles (0..7)
ODR = OT - OBF  # 3 o-tiles in fp8 DoubleRow
O_BF = OBF * P  # 1024 bf16 output columns
O_DR = ODR * P  # 384 DR output columns
KT = IN // P  # 32 k-tiles (== quant groups, GROUP == P)
KTP = KT // 2  # 16 DoubleRow k-pairs
TCH = 512  # tokens per chunk == PSUM bank free size (f32)
NT = TOKENS // TCH  # 4 chunks
NB = 8  # psum banks
WAVE = 8  # wave-1 groups (tt=0, all bf16 o-tiles)
XB = 2  # x chunk buffers
PWB = 6  # packed wave staging slots
PDB = 8  # packed DR staging slots
PD_ILV = 32  # no pd interleave: the DR pack streams after x(1)
NYS = 4  # y eviction slots

PW_BYTES = O_BF + 2 * O_BF  # 3072: w8 | sc bf16
PD_BYTES = O_DR + 2 * O_DR  # 1152: w8 | sc bf16

# fp8 scaling: W8 = e4m3(w*sc*2^9), x8 = e4m3(x*(1/s)*2^4)
WSH = 512.0
XSH = 16.0
EV_SCALE = 2.0 ** -13

# post-wave schedule: chunk-1 bf16 runs right after the wave (its x
# lands just as the wave ends), then the chunk-0 and chunk-1 DR tiles
# (their dequant input streams during chunk-1 bf16), then chunks 2..3
# as bf16-then-DR. (c0's bf16 tiles all ran in wave 1.)
SCHED = [("bf", 1, _ot) for _ot in range(OBF)]
SCHED += [("dr", 0, _j) for _j in range(ODR)]
SCHED += [("dr", 1, _j) for _j in range(ODR)]
for _tt in range(2, NT):
    SCHED += [("bf", _tt, _ot) for _ot in range(OBF)]
    SCHED += [("dr", _tt, _j) for _j in range(ODR)]
NENT = WAVE + len(SCHED)  # 8 + 36 = 44 evict entries

# matmul count after each entry's last matmul (wave entries are
# kc-major interleaved: group b's last mm is 8*(KT-1)+b+1)
MM_END = [WAVE * (KT - 1) + b + 1 for b in range(WAVE)]
_cum = WAVE * KT  # 256
MM_WAVE_END = _cum
MM_C0_END = MM_C1BF_END = MM_C1_END = None
for _kind, _tt, _i in SCHED:
    _cum += KT if _kind == "bf" else KTP
    MM_END.append(_cum)
    if _tt == 0:
        MM_C0_END = _cum
    if _tt == 1 and _kind == "bf":
        MM_C1BF_END = _cum
    if _tt == 1:
        MM_C1_END = _cum

# ---- DVE (vector engine) program order and s_dq values ----
# recip, inv16, then per kc: bf-stt(kc) (+ dr-stt(kc-PD_ILV) once the
# interleaved DR pack pieces land), then the dr-stt tail, then
# conv-c1/2/3 blocks. The c0 x8 conversions run on the Scalar engine
# (s_cv0) - with them on the DVE the wave becomes DVE-paced (bf-stt
# [128,1024] is ~1.29us; no DVE 2x mode applies because the fp8
# operand is 1-byte).
DQ_BF = {}
DQ_DR = {}
_ctr = 2  # 1 recip, 2 inv16
for _kc in range(KT):
    _ctr += 1
    DQ_BF[_kc] = _ctr
    if _kc >= PD_ILV:
        _ctr += 1
        DQ_DR[_kc - PD_ILV] = _ctr
for _j in range(KT - PD_ILV, KT):
    _ctr += 1
    DQ_DR[_j] = _ctr
DQ_ALL_DR = _ctr  # 66: w8d2 fully dequanted


def dq_conv(tt):
    return DQ_ALL_DR + KT * tt  # chunk-tt x8 conversions done


LAST = {}  # exec_time_ns etc. for the local test harness

_NC_CACHE = {}


def _build_nc():
    import concourse.bass as bass
    from concourse import mybir

    f32 = mybir.dt.float32
    bf16 = mybir.dt.bfloat16
    fp8 = mybir.dt.float8e4
    u8 = mybir.dt.uint8

    nc = bass.Bass()
    pack_w = nc.declare_dram_parameter("pack_w", [IN, PW_BYTES], u8, isOutput=False)
    pack_d = nc.declare_dram_parameter("pack_d", [IN, PD_BYTES], u8, isOutput=False)
    xT = nc.declare_dram_parameter("xT", [IN, TOKENS], bf16, isOutput=False)
    s_cols = nc.declare_dram_parameter("s_cols", [P, KT], f32, isOutput=False)
    bias_cols = nc.declare_dram_parameter("bias_cols", [P, OBF], f32, isOutput=False)
    bias_dr = nc.declare_dram_parameter("bias_dr", [P, ODR], f32, isOutput=False)
    yT = nc.declare_dram_parameter("yT", [O_PAD, TOKENS], bf16, isOutput=True)

    with ExitStack() as ctx:
        w_all = ctx.enter_context(nc.sbuf_tensor("w_all", [P, KT * O_BF], bf16))
        w8d2 = ctx.enter_context(nc.sbuf_tensor("w8d2", [P, KTP, 2, O_DR], fp8))
        xn_all = ctx.enter_context(nc.sbuf_tensor("xn_all", [P, XB * KT * TCH], bf16))
        x8 = ctx.enter_context(nc.sbuf_tensor("x8", [P, XB * KT, TCH], fp8))
        y_sl = ctx.enter_context(nc.sbuf_tensor("y_sl", [P, NYS * TCH], bf16))
        pw = ctx.enter_context(nc.sbuf_tensor("pw", [P, PWB * PW_BYTES], u8))
        pd = ctx.enter_context(nc.sbuf_tensor("pd", [P, PDB * PD_BYTES], u8))
        s_sb = ctx.enter_context(nc.sbuf_tensor("s_sb", [P, KT], f32))
        inv_s = ctx.enter_context(nc.sbuf_tensor("inv_s", [P, KT], f32))
        inv16 = ctx.enter_context(nc.sbuf_tensor("inv16", [P, KT], f32))
        bias_sb = ctx.enter_context(nc.sbuf_tensor("bias_sb", [P, OBF], f32))
        bias_dsb = ctx.enter_context(nc.sbuf_tensor("bias_dsb", [P, ODR], f32))
        ps = [
            ctx.enter_context(nc.psum_tensor(f"ps{i}", [P, TCH], f32))
            for i in range(NB)
        ]
        s_tbl = ctx.enter_context(nc.semaphore("s_tbl"))
        s_wd = [ctx.enter_context(nc.semaphore(f"s_wd{k}")) for k in range(KT)]
        s_wdr = [ctx.enter_context(nc.semaphore(f"s_wdr{k}")) for k in range(KT)]
        s_xc = [ctx.enter_context(nc.semaphore(f"s_xc{t}")) for t in range(1, NT)]
        s_dq = ctx.enter_context(nc.semaphore("s_dq"))
        s_cv0 = ctx.enter_context(nc.semaphore("s_cv0"))
        s_pe = ctx.enter_context(nc.semaphore("s_pe"))
        s_act = ctx.enter_context(nc.semaphore("s_act"))
        s_ys = [ctx.enter_context(nc.semaphore(f"s_ys{j}")) for j in range(NYS)]
        block = ctx.enter_context(nc.Block())

        def wd(kc):
            return w_all[:, kc * O_BF : (kc + 1) * O_BF]

        def pw_slot(kc):
            o = (kc % PWB) * PW_BYTES
            return pw[:, o : o + PW_BYTES]

        def pw_w8(kc):
            o = (kc % PWB) * PW_BYTES
            return pw[:, o : o + O_BF].bitcast(mybir.dt.float8e4)

        def pw_sc(kc):
            o = (kc % PWB) * PW_BYTES
            return pw[:, o + O_BF : o + 3 * O_BF].bitcast(bf16)

        def pd_slot(kc):
            o = (kc % PDB) * PD_BYTES
            return pd[:, o : o + PD_BYTES]

        def pd_w8(kc):
            o = (kc % PDB) * PD_BYTES
            return pd[:, o : o + O_DR].bitcast(mybir.dt.float8e4)

        def pd_sc(kc):
            o = (kc % PDB) * PD_BYTES
            return pd[:, o + O_DR : o + PD_BYTES].bitcast(bf16)

        def xreg(tt, kc):
            o = ((tt % XB) * KT + kc) * TCH
            return xn_all[:, o : o + TCH]

        def x8reg(tt, kc):
            return x8[:, (tt % XB) * KT + kc, :]

        def x8pair(tt, kp):
            b = (tt % XB) * KT
            return x8[:, b + 2 * kp : b + 2 * kp + 2, :]

        def yslot(e):
            o = (e % NYS) * TCH
            return y_sl[:, o : o + TCH]

        @block.sync
        def _(sync):
            def xchunk(tt):
                o = (tt % XB) * KT * TCH
                sync.dma_start(
                    out=xn_all[:, o : o + KT * TCH],
                    in_=xT[:, tt * TCH : (tt + 1) * TCH].rearrange(
                        "(kc p) t -> p kc t", kc=KT
                    ),
                ).then_inc(s_xc[tt - 1], 16)

            sync.dma_start(out=s_sb[:, :], in_=s_cols[:, :]).then_inc(s_tbl, 16)
            # wave stream: per kc a packed [w8-bf | sc-bf] transfer plus
            # the x(0) piece (both count s_wd[kc], terminal 32); DR pack
            # pieces interleave from kc >= PD_ILV. Slot WAR gates target
            # the DVE counter (bf-stt / dr-stt are the only readers).
            for kc in range(KT):
                if kc >= PWB:
                    sync.wait_ge(s_dq, DQ_BF[kc - PWB])
                sync.dma_start(
                    out=pw_slot(kc), in_=pack_w[kc * P : (kc + 1) * P, :]
                ).then_inc(s_wd[kc], 16)
                sync.dma_start(
                    out=xreg(0, kc), in_=xT[kc * P : (kc + 1) * P, 0:TCH]
                ).then_inc(s_wd[kc], 16)
                if kc == 26:
                    sync.dma_start(out=bias_sb[:, :], in_=bias_cols[:, :]).then_inc(
                        s_tbl, 16
                    )
                    sync.dma_start(out=bias_dsb[:, :], in_=bias_dr[:, :]).then_inc(
                        s_tbl, 16
                    )
                if kc >= PD_ILV:
                    j = kc - PD_ILV
                    if j >= PDB:
                        sync.wait_ge(s_dq, DQ_DR[j - PDB])
                    sync.dma_start(
                        out=pd_slot(j), in_=pack_d[j * P : (j + 1) * P, :]
                    ).then_inc(s_wdr[j], 16)
            def ystore(e):
                sync.wait_ge(s_act, e + 1)
                if e < WAVE:
                    tt, i = 0, e
                else:
                    kind, tt, i = SCHED[e - WAVE]
                    if kind == "dr":
                        i += OBF
                sync.dma_start(
                    out=yT[i * P : (i + 1) * P, tt * TCH : (tt + 1) * TCH],
                    in_=yslot(e),
                ).then_inc(s_ys[e % NYS], 16)

            # x(1): needed when chunk-1 bf16 starts, right at wave end.
            # x chunks 1..3 are terminal-gated, so each ships as ONE
            # batched DMA ([(kc p) t] -> [p, kc, t]) - 1 SP issue instead
            # of 32 (the SP sequencer spends 565ns per dma_start).
            xchunk(1)
            for e in range(WAVE):  # wave stores
                ystore(e)
            # DR pack: its dequant is only needed once chunk-1 bf16 ends
            for j in range(KT):
                if j >= PDB:
                    sync.wait_ge(s_dq, DQ_DR[j - PDB])
                sync.dma_start(
                    out=pd_slot(j), in_=pack_d[j * P : (j + 1) * P, :]
                ).then_inc(s_wdr[j], 16)

            # x(2) reuses xn slot 0: wave matmuls and c0 conversions must
            # have retired (both single-producer counters). It must issue
            # before the c0/c1 DR stores (their evicts land much later).
            sync.wait_ge(s_pe, MM_WAVE_END)
            sync.wait_ge(s_cv0, KT)
            xchunk(2)
            for e in range(WAVE, WAVE + OBF + 2 * ODR):  # c1-bf + c0/c1 DR
                ystore(e)
            # x(3) reuses xn slot 1: chunk-1 bf16 matmuls and the c1
            # conversions must have retired.
            sync.wait_ge(s_pe, MM_C1BF_END)
            sync.wait_ge(s_dq, dq_conv(1))
            xchunk(3)
            for e in range(WAVE + OBF + 2 * ODR, NENT):
                ystore(e)
            for j in range(NYS):
                sync.wait_ge(s_ys[j], 16 * (NENT // NYS))

        @block.vector
        def _(vector):
            vector.wait_ge(s_tbl, 16)  # s_cols landed
            nc.vector.reciprocal(out=inv_s[:, :], in_=s_sb[:, :]).then_inc(s_dq, 1)
            vector.wait_ge(s_dq, 1)  # recip retired before reads of inv_s
            nc.vector.tensor_scalar_mul(inv16[:, :], inv_s[:, :], XSH).then_inc(
                s_dq, 1
            )
            vector.wait_ge(s_dq, 2)

            def dr_stt(j):
                vector.wait_ge(s_wdr[j], 16)
                nc.vector.scalar_tensor_tensor(
                    w8d2[:, j // 2, j % 2, :],
                    pd_w8(j),
                    WSH,
                    pd_sc(j),
                    mybir.AluOpType.mult,
                    mybir.AluOpType.mult,
                ).then_inc(s_dq, 1)

            # per kc: dequant bf16 W' and (once the interleaved DR pack
            # pieces land) dequant DR columns
            for kc in range(KT):
                vector.wait_ge(s_wd[kc], 32)
                nc.vector.scalar_tensor_tensor(
                    wd(kc),
                    pw_w8(kc),
                    inv_s[:, kc : kc + 1],
                    pw_sc(kc),
                    mybir.AluOpType.mult,
                    mybir.AluOpType.mult,
                ).then_inc(s_dq, 1)
                if kc >= PD_ILV:
                    dr_stt(kc - PD_ILV)
            for j in range(KT - PD_ILV, KT):
                dr_stt(j)
            # x -> fp8 conversions for chunks 1..3. s_xc is a bulk-chunk
            # counter whose DMA completions may reorder, so only its
            # terminal value is meaningful.
            for tt in range(1, NT):
                if tt == 2:
                    vector.wait_ge(s_pe, MM_C0_END)  # x8 slot 0 free
                if tt == 3:
                    vector.wait_ge(s_pe, MM_C1_END)  # x8 slot 1 free
                vector.wait_ge(s_xc[tt - 1], 16)
                for kc in range(KT):
                    nc.vector.tensor_scalar_mul(
                        x8reg(tt, kc), xreg(tt, kc), inv16[:, kc : kc + 1]
                    ).then_inc(s_dq, 1)

        @block.scalar
        def _(scalar):
            # chunk-0 x8 conversions (x8 = e4m3(x * 16/s)) run here so the
            # DVE wave pipeline stays arrival-paced.
            scalar.wait_ge(s_dq, 2)  # inv16 ready
            for kc in range(KT):
                scalar.wait_ge(s_wd[kc], 32)
                nc.scalar.activation(
                    x8reg(0, kc),
                    xreg(0, kc),
                    mybir.ActivationFunctionType.Copy,
                    bias=0.0,
                    scale=inv16[:, kc : kc + 1],
                ).then_inc(s_cv0, 1)
            scalar.wait_ge(s_tbl, 48)
            for e in range(NENT):
                if e >= NYS:
                    scalar.wait_ge(s_ys[e % NYS], 16 * (e // NYS))
                scalar.wait_ge(s_pe, MM_END[e])
                if e < WAVE:
                    kind, i = "bf", e
                else:
                    kind, _tt, i = SCHED[e - WAVE]
                if kind == "bf":
                    nc.scalar.activation(
                        yslot(e),
                        ps[e % NB][:, :],
                        mybir.ActivationFunctionType.Identity,
                        bias=bias_sb[:, i : i + 1],
                        scale=1.0,
                    ).then_inc(s_act, 1)
                else:
                    nc.scalar.activation(
                        yslot(e),
                        ps[e % NB][:, :],
                        mybir.ActivationFunctionType.Identity,
                        bias=bias_dsb[:, i : i + 1],
                        scale=EV_SCALE,
                    ).then_inc(s_act, 1)

        @block.tensor
        def _(tensor):
            DR = mybir.MatmulPerfMode.DoubleRow
            # wave 1: groups (tt=0, ot=0..7) accumulate kc-major.
            for kc in range(KT):
                tensor.wait_ge(s_wd[kc], 32)  # x(0) piece landed
                tensor.wait_ge(s_dq, DQ_BF[kc])  # W'(kc) dequanted
                for b in range(WAVE):
                    nc.tensor.matmul(
                        ps[b][:, :],
                        wd(kc)[:, b * P : (b + 1) * P],
                        xreg(0, kc),
                        start=(kc == 0),
                        stop=(kc == KT - 1),
                    ).then_inc(s_pe, 1)
            # post-wave entries, sequential. DR tiles are full 128-row
            # DoubleRow groups: 16 pair-matmuls cover all 32 k-tiles.
            for ei, (kind, tt, i) in enumerate(SCHED):
                e = WAVE + ei
                if kind == "bf":
                    if i == 0:
                        tensor.wait_ge(s_xc[tt - 1], 16)
                else:
                    if i == 0:
                        if tt == 0:
                            # chunk-0 DR starts right at wave end and
                            # chases the dr-stt tail with per-pair waits
                            tensor.wait_ge(s_cv0, KT)
                        else:
                            # w8d2 complete and chunk-tt x8 ready
                            tensor.wait_ge(s_dq, dq_conv(tt))
                tensor.wait_ge(s_act, e - NB + 1)  # psum bank recycled
                if kind == "bf":
                    for kc in range(KT):
                        nc.tensor.matmul(
                            ps[e % NB][:, :],
                            wd(kc)[:, i * P : (i + 1) * P],
                            xreg(tt, kc),
                            start=(kc == 0),
                            stop=(kc == KT - 1),
                        ).then_inc(s_pe, 1)
                else:
                    for kp in range(KTP):
                        if tt == 0 and i == 0:
                            tensor.wait_ge(s_dq, DQ_DR[2 * kp + 1])
                        nc.tensor.matmul(
                            ps[e % NB][:, :],
                            w8d2[:, kp, :, i * P : (i + 1) * P],
                            x8pair(tt, kp),
                            start=(kp == 0),
                            stop=(kp == KTP - 1),
                            perf_mode=DR,
                        ).then_inc(s_pe, 1)

    return nc


def get_nc():
    if "nc" not in _NC_CACHE:
        _NC_CACHE["nc"] = _build_nc()
    return _NC_CACHE["nc"]


def _prep_inputs(x, w_q, scales, s, bias):
    import ml_dtypes

    bf16 = ml_dtypes.bfloat16
    fp8 = ml_dtypes.float8_e4m3
    x = np.asarray(x, dtype=np.float32)
    w_q = np.asarray(w_q)
    scales = np.asarray(scales, dtype=np.float32)
    s = np.asarray(s, dtype=np.float32)
    bias = np.asarray(bias, dtype=np.float32)

    pad = O_PAD - O_SHARD  # 32 rows of zero-padding per shard
    w = w_q.reshape(OUT, IN).astype(fp8)  # ints in [-7,7] -> exact
    sc = scales.reshape(OUT, N_GROUPS)  # f32

    xT = np.ascontiguousarray(x.T.astype(bf16))  # [IN, TOKENS] bf16
    s_cols = np.ascontiguousarray(s.reshape(KT, P).T)  # [128, 32] f32

    in_maps = []
    for c in range(N_CORES):
        lo, hi = c * O_SHARD, (c + 1) * O_SHARD
        w_c = np.pad(w[lo:hi], ((0, pad), (0, 0)))  # [O_PAD, IN] fp8
        sc_c = np.pad(sc[lo:hi], ((0, pad), (0, 0)))  # [O_PAD, 32]
        b_c = np.pad(bias[lo:hi], (0, pad))  # [O_PAD]
        w8T = np.ascontiguousarray(w_c.T)  # [IN, O_PAD] fp8
        # scales replicated per k-row (layout move): row k holds
        # sc_c[:, k//128] in bf16
        scT = np.ascontiguousarray(
            np.repeat(sc_c.T.astype(bf16), P, axis=0)
        )  # [IN, O_PAD] bf16
        pack_w = np.concatenate(
            [
                w8T[:, :O_BF].view(np.uint8),
                np.ascontiguousarray(scT[:, :O_BF]).view(np.uint8),
            ],
            axis=1,
        )  # [IN, 3072] u8
        pack_d = np.concatenate(
            [
                w8T[:, O_BF:].view(np.uint8),
                np.ascontiguousarray(scT[:, O_BF:]).view(np.uint8),
            ],
            axis=1,
        )  # [IN, 1152] u8
        in_maps.append(
            {
                "pack_w": np.ascontiguousarray(pack_w),
                "pack_d": np.ascontiguousarray(pack_d),
                "xT": xT,
                "s_cols": s_cols,
                "bias_cols": np.ascontiguousarray(
                    b_c[:O_BF].reshape(OBF, P).T
                ),  # [128, 8] f32
                "bias_dr": np.ascontiguousarray(
                    b_c[O_BF:].reshape(ODR, P).T
                ),  # [128, 3] f32
            }
        )
    return in_maps


def _install_profile_shim():
    """Provide antenv.axon_hooks (NTFF profiling via libaxon ctypes) when
    the container image lacks it. Only used for local perf iteration."""
    import contextlib
    import ctypes
    import sys
    import types

    if "antenv.axon_hooks" in sys.modules:
        return
    so_path = "/opt/axon/libaxon_pjrt.so"
    try:
        lib = ctypes.CDLL(so_path)
    except OSError:
        return
    if not hasattr(lib, "axon_start_nrt_profile"):
        return
    lib.axon_start_nrt_profile.argtypes = [
        ctypes.POINTER(ctypes.c_int64),
        ctypes.c_size_t,
    ]
    lib.axon_start_nrt_profile.restype = ctypes.c_int64
    lib.axon_stop_nrt_profile.argtypes = [ctypes.c_char_p]
    lib.axon_stop_nrt_profile.restype = ctypes.c_int64

    @contextlib.contextmanager
    def _hook(output_dir, device_ids):
        import jax

        jax.devices()
        if device_ids:
            ids = (ctypes.c_int64 * len(device_ids))(*device_ids)
            rc = lib.axon_start_nrt_profile(ids, len(device_ids))
        else:
            rc = lib.axon_start_nrt_profile(None, 0)
        if rc != 0:
            raise RuntimeError(f"axon_start_nrt_profile rc={rc}")
        try:
            yield
        finally:
            n = lib.axon_stop_nrt_profile(str(output_dir).encode())
            print(f"profile: {n} file(s) written to {output_dir}", file=sys.stderr)

    mod = types.ModuleType("antenv.axon_hooks")
    mod.get_axon_ntff_profile_hook = lambda: _hook
    mod.set_axon_ntff_profile_hook = lambda h: None
    sys.modules["antenv.axon_hooks"] = mod


def kernel(x, w_q, scales, s, bias):
    import sys

    if "/opt/trn_rl_repo" not in sys.path:
        sys.path.insert(0, "/opt/trn_rl_repo")
    import concourse.bass_utils as bass_utils
    from concourse.bass_utils import run_bass_kernel_spmd

    orig_dtype = np.asarray(x).dtype
    in_maps = _prep_inputs(x, w_q, scales, s, bias)
    nc = get_nc()

    trace = bool(os.environ.get("AWQ_TRACE"))
    kwargs = {}
    if trace:
        _install_profile_shim()
        bass_utils.upload_artifacts = lambda d: d  # zero-egress container
        tmpdir = os.environ.get("AWQ_TRACE_DIR")
        if tmpdir:
            os.makedirs(tmpdir, exist_ok=True)
            kwargs["tmpdir"] = tmpdir
    res = run_bass_kernel_spmd(
        nc,
        in_maps,
        core_ids=list(range(N_CORES)),
        trace=trace,
        **kwargs,
    )
    LAST["exec_time_ns"] = res.exec_time_ns
    LAST["results"] = res

    yT_full = np.concatenate(
        [np.asarray(res.results[c]["yT"], dtype=np.float32) for c in range(N_CORES)],
        axis=0,
    )  # [8*1408, 2048] f32
    y = np.ascontiguousarray(
        yT_full.reshape(N_CORES, O_PAD, TOKENS)[:, :O_SHARD, :]
        .reshape(OUT, TOKENS)
        .T
    )
    return y.astype(orig_dtype)
